# revision 3
# baseline (speedup 1.0000x reference)
"""TRN2 Bass kernel for nn_ODEModel (RK4 neural ODE, dense MLP vector field).

Strategy: pure DATA-parallel over the batch (256/8 = 32 samples per core),
all MLP weights replicated and SBUF-resident -> ZERO collectives (the
baseline spent ~90% of its time in 64 sequential AllGathers).

Per f-eval (64 sequential evals = 16 RK4 steps x 4 stages), per core:
  h1  = relu(W1 y + b1)        dim-major [4096, 32]; W1 chunks stationary
                               (128 cols -> FWL), y [5,32] moving.
  h2b = relu(h1^T W2^T + b2)   batch-major [32, 2048] via 4 psum tiles
                               [32,512]: h1 chunk [128,32] STATIONARY
                               (27ns LDW, hidden), W2^T chunk [128,512]
                               MOVING (N=512 amortizes best, ~131ns/MM).
                               b2 added with a K=1 rank-1 matmul.
  h2t = transpose(h2b)         16 PE transposes [32,128] -> dim-major
  z   = W3 h2t + b3; tanh      16 accumulating MMs into [4,32] psum
  k   = tanh + poly(y)         poly via 3 tiny matmuls on quad features
RK4 combination in fp32 exactly as the reference; matmul operands bf16,
PSUM accumulation fp32.  Output [16,4,32] written dim-major, host
transposes/assembles.
"""
import sys

sys.path.insert(0, "/opt/trn_rl_repo")
import numpy as np
import ml_dtypes

import concourse.bass as bass
import concourse.bacc as bacc
import concourse.tile as tile
import concourse.mybir as mybir

F32 = mybir.dt.float32
BF16 = mybir.dt.bfloat16
NP_BF16 = ml_dtypes.bfloat16

N_CORES = 8
B_FULL = 256
BS = B_FULL // N_CORES   # 32 samples per core
D = 4
H1 = 4096
H2 = 2048
K1 = H1 // 128           # 32 h1 chunks (contraction for L2)
N2 = H2 // 512           # 4 L2 output chunks of 512 h2 dims
M3 = H2 // 128           # 16 L3 contraction chunks


def build_dp(hs):
    T1 = len(hs)
    nc = bacc.Bacc("TRN2", target_bir_lowering=False, debug=False,
                   num_devices=N_CORES)

    d_y0T = nc.dram_tensor("y0T", [5, BS], F32, kind="ExternalInput").ap()
    d_y0b = nc.dram_tensor("y0b", [5, BS], BF16, kind="ExternalInput").ap()
    d_w1b = nc.dram_tensor("w1b", [5, H1], BF16, kind="ExternalInput").ap()
    d_w2t = nc.dram_tensor("w2t", [128, K1 * H2], BF16, kind="ExternalInput").ap()
    d_b2r = nc.dram_tensor("b2r", [1, H2], BF16, kind="ExternalInput").ap()
    d_one = nc.dram_tensor("one1", [1, BS], BF16, kind="ExternalInput").ap()
    d_w3t = nc.dram_tensor("w3t", [128, M3 * D], BF16, kind="ExternalInput").ap()
    d_b3c = nc.dram_tensor("b3c", [D, 1], F32, kind="ExternalInput").ap()
    d_wpa = nc.dram_tensor("wpa", [5, D], BF16, kind="ExternalInput").ap()
    d_wpbs = nc.dram_tensor("wpbs", [D, D], BF16, kind="ExternalInput").ap()
    d_wpbc = nc.dram_tensor("wpbc", [3, D], BF16, kind="ExternalInput").ap()
    d_i32 = nc.dram_tensor("i32", [BS, BS], BF16, kind="ExternalInput").ap()
    d_out = nc.dram_tensor("out", [D, T1 * BS], F32, kind="ExternalOutput").ap()

    with tile.TileContext(nc) as tc:
        with tc.tile_pool(name="wpool", bufs=1) as wp, \
             tc.tile_pool(name="state", bufs=1) as stp, \
             tc.tile_pool(name="act", bufs=2) as actp, \
             tc.tile_pool(name="small", bufs=3) as smp, \
             tc.tile_pool(name="ps_l1", bufs=2, space="PSUM") as ps_l1, \
             tc.tile_pool(name="ps_l2", bufs=2, space="PSUM") as ps_l2, \
             tc.tile_pool(name="ps_tr", bufs=2, space="PSUM") as ps_tr, \
             tc.tile_pool(name="ps_sm", bufs=2, space="PSUM") as ps_sm:

            w1b = wp.tile([5, H1], BF16)
            w2t = wp.tile([128, K1 * H2], BF16)
            b2r = wp.tile([1, H2], BF16)
            one1 = wp.tile([1, BS], BF16)
            w3t = wp.tile([128, M3 * D], BF16)
            b3c = wp.tile([D, 1], F32)
            wpa = wp.tile([5, D], BF16)
            wpbs = wp.tile([D, D], BF16)
            wpbc = wp.tile([3, D], BF16)
            i32 = wp.tile([BS, BS], BF16)
            for t_, d_ in ((w1b, d_w1b), (b2r, d_b2r), (one1, d_one),
                           (w3t, d_w3t), (b3c, d_b3c), (wpa, d_wpa),
                           (wpbs, d_wpbs), (wpbc, d_wpbc), (i32, d_i32)):
                nc.sync.dma_start(t_[:], d_)
            # big W2^T load split across 4 DMAs for queue parallelism
            q4 = K1 * H2 // 4
            for j in range(4):
                nc.sync.dma_start(w2t[:, j * q4:(j + 1) * q4],
                                  d_w2t[:, j * q4:(j + 1) * q4])

            yb16 = stp.tile([5, BS], BF16, name="yb16g")
            nc.sync.dma_start(yb16[:], d_y0b)
            ybase = smp.tile([D, BS], F32, name="ybaseg", tag="ybase")
            nc.sync.dma_start(ybase[:], d_y0T[0:4, :])

            A = mybir.AluOpType
            RELU = mybir.ActivationFunctionType.Relu
            TANH = mybir.ActivationFunctionType.Tanh

            def emit_eval(c, racc):
                """One f-eval + the early half of the RK4 combination.
                Returns (poly_ps, th, z)."""
                # ---- poly features (off critical path; z needed only at end)
                yshb = actp.tile([3, BS], BF16, name="yshbg", tag="yshb")
                nc.sync.dma_start(yshb[:], yb16[1:4, :])
                phis = actp.tile([D, BS], BF16, name="phisg", tag="phis")
                phic = actp.tile([3, BS], BF16, name="phicg", tag="phic")
                nc.gpsimd.tensor_mul(phis[:], yb16[0:4, :], yb16[0:4, :])
                nc.gpsimd.tensor_mul(phic[:], yb16[0:3, :], yshb[:])
                poly_ps = ps_sm.tile([D, BS], F32, name="polyg", tag="sm")
                nc.tensor.matmul(poly_ps[:], wpa[:], yb16[:], start=True, stop=False)
                nc.tensor.matmul(poly_ps[:], wpbs[:], phis[:], start=False, stop=False)
                nc.tensor.matmul(poly_ps[:], wpbc[:], phic[:], start=False, stop=True)

                # ---- z = ybase + c*poly (+ c*racc at stage 3): early, DVE
                z = smp.tile([D, BS], F32, name="zg", tag="z")
                if racc is None:
                    nc.vector.scalar_tensor_tensor(
                        z[:], poly_ps[:], c, ybase[:], op0=A.mult, op1=A.add)
                else:
                    zr = smp.tile([D, BS], F32, name="zrg", tag="zr")
                    nc.vector.scalar_tensor_tensor(
                        zr[:], racc[:], c, ybase[:], op0=A.mult, op1=A.add)
                    nc.vector.scalar_tensor_tensor(
                        z[:], poly_ps[:], c, zr[:], op0=A.mult, op1=A.add)

                # ---- L1: h1 = relu(W1 y + b1), dim-major [128, K1*32]
                h1b = actp.tile([128, K1 * BS], BF16, name="h1bg", tag="h1b")
                for g in range(2):
                    h1ps = ps_l1.tile([128, 512], F32, name="h1ps", tag="h1ps")
                    for q in range(16):
                        m = g * 16 + q
                        nc.tensor.matmul(h1ps[:, q * BS:(q + 1) * BS],
                                         w1b[:, m * 128:(m + 1) * 128],
                                         yb16[:], start=True, stop=True)
                    nc.scalar.activation(h1b[:, g * 512:(g + 1) * 512],
                                         h1ps[:], RELU)

                # ---- L2: h2b[b, d] batch-major; h1 chunks stationary,
                #      W2^T moving (N=512).  Transposes deferred one n-chunk
                #      so relu_n hides under n+1's matmuls.
                h2b = actp.tile([BS, H2], BF16, name="h2bg", tag="h2b")
                h2t = actp.tile([128, M3 * BS], BF16, name="h2tg", tag="h2t")

                def emit_tr(n):
                    trp = ps_tr.tile([128, 128], BF16, name="trp", tag="trp")
                    for j in range(4):
                        m = n * 4 + j
                        nc.tensor.transpose(trp[:, j * BS:(j + 1) * BS],
                                            h2b[:, m * 128:(m + 1) * 128],
                                            i32[:])
                    nc.scalar.copy(h2t[:, n * 128:(n + 1) * 128], trp[:])

                for n in range(N2):
                    h2ps = ps_l2.tile([BS, 512], F32, name="h2ps", tag="h2ps")
                    nc.tensor.matmul(h2ps[:], one1[:],
                                     b2r[:, n * 512:(n + 1) * 512],
                                     start=True, stop=False)
                    for k in range(K1):
                        nc.tensor.matmul(
                            h2ps[:],
                            h1b[:, k * BS:(k + 1) * BS],
                            w2t[:, k * H2 + n * 512:k * H2 + (n + 1) * 512],
                            start=False, stop=(k == K1 - 1))
                    nc.scalar.activation(h2b[:, n * 512:(n + 1) * 512],
                                         h2ps[:], RELU)
                    if n > 0:
                        emit_tr(n - 1)
                emit_tr(N2 - 1)

                # ---- L3: z3 = W3 h2 + b3 (bias via tanh activation)
                z_ps = ps_sm.tile([D, BS], F32, name="zpsg", tag="sm")
                for m in range(M3):
                    nc.tensor.matmul(z_ps[:], w3t[:, m * D:(m + 1) * D],
                                     h2t[:, m * BS:(m + 1) * BS],
                                     start=(m == 0), stop=(m == M3 - 1))
                th = smp.tile([D, BS], F32, name="thg", tag="th")
                nc.scalar.activation(th[:], z_ps[:], TANH, bias=b3c[:, 0:1])
                return poly_ps, th, z

            racc = None
            for t in range(T1):
                h = float(hs[t])
                cs = [h / 2, h / 2, h, h / 6]
                for stage in range(4):
                    c = cs[stage]
                    poly_ps, th, z = emit_eval(c, racc if stage == 3 else None)
                    # critical op: next eval's input
                    nc.vector.scalar_tensor_tensor(
                        yb16[0:4, :], th[:], c, z[:], op0=A.mult, op1=A.add)
                    if stage < 3:
                        k_sb = smp.tile([D, BS], F32, name="kg", tag="k")
                        nc.vector.tensor_add(k_sb[:], th[:], poly_ps[:])
                        if stage == 0:
                            racc = k_sb
                        else:
                            r = smp.tile([D, BS], F32, name="raccg", tag="racc")
                            nc.vector.scalar_tensor_tensor(
                                r[:], k_sb[:], 2.0, racc[:],
                                op0=A.mult, op1=A.add)
                            racc = r
                    else:
                        ynew = smp.tile([D, BS], F32, name="ybaseg", tag="ybase")
                        nc.vector.scalar_tensor_tensor(
                            ynew[:], th[:], c, z[:], op0=A.mult, op1=A.add)
                        ybase = ynew
                        nc.sync.dma_start(d_out[:, t * BS:(t + 1) * BS], ynew[:])
    nc.compile()
    return nc


def prep_inputs_dp(s_grid, y0, W1, b1, W2, b2, W3, b3, wpoly):
    hs = np.diff(np.asarray(s_grid, np.float64)).astype(np.float32)
    w1bm = np.concatenate([np.asarray(W1, np.float32).T,
                           np.asarray(b1, np.float32)[None, :]], 0).astype(NP_BF16)
    W2a = np.asarray(W2, np.float32)           # [2048, 4096]
    w2tm = np.ascontiguousarray(
        W2a.T.reshape(K1, 128, H2).transpose(1, 0, 2).reshape(128, K1 * H2)
    ).astype(NP_BF16)
    b2r = np.asarray(b2, np.float32)[None, :].astype(NP_BF16)
    one1 = np.ones((1, BS), np.float32).astype(NP_BF16)
    W3a = np.asarray(W3, np.float32)           # [4, 2048]
    w3tm = np.ascontiguousarray(
        W3a.T.reshape(M3, 128, D).transpose(1, 0, 2).reshape(128, M3 * D)
    ).astype(NP_BF16)
    b3c = np.asarray(b3, np.float32)[:, None]
    w = np.asarray(wpoly, np.float32)
    wpa = np.zeros((5, 4), np.float32)
    wpb = np.zeros((7, 4), np.float32)
    wpa[4, 0] = w[0]; wpa[0, 0] = w[1]; wpb[0, 0] = w[2]
    wpa[4, 1] = w[3]; wpa[0, 1] = w[4]; wpb[0, 1] = w[5]
    wpa[1, 1] = w[6]; wpb[1, 1] = w[7]; wpb[4, 1] = w[8]
    wpa[4, 2] = w[9]; wpa[2, 2] = w[10]; wpb[2, 2] = w[11]
    wpa[1, 2] = w[12]; wpb[1, 2] = w[13]; wpb[5, 2] = w[14]
    wpa[4, 3] = w[15]; wpa[3, 3] = w[16]; wpb[3, 3] = w[17]
    wpa[2, 3] = w[18]; wpb[2, 3] = w[19]; wpb[6, 3] = w[20]
    wpbs = wpb[0:4].astype(NP_BF16)
    wpbc = wpb[4:7].astype(NP_BF16)
    wpa = wpa.astype(NP_BF16)
    i32 = np.eye(BS, dtype=np.float32).astype(NP_BF16)
    y0a = np.asarray(y0, np.float32)
    in_maps = []
    for cidx in range(N_CORES):
        y0T5 = np.concatenate([y0a[cidx * BS:(cidx + 1) * BS].T,
                               np.ones((1, BS), np.float32)], 0)
        y0T5 = np.ascontiguousarray(y0T5)
        in_maps.append({
            "y0T": y0T5, "y0b": y0T5.astype(NP_BF16), "w1b": w1bm,
            "w2t": w2tm, "b2r": b2r, "one1": one1, "w3t": w3tm, "b3c": b3c,
            "wpa": wpa, "wpbs": wpbs, "wpbc": wpbc, "i32": i32,
        })
    return hs, in_maps


def assemble_dp(results, y0):
    y0a = np.asarray(y0, np.float32)
    T1 = results[0]["out"].shape[1] // BS
    out = np.empty((T1 + 1, B_FULL, D), np.float32)
    out[0] = y0a
    for c in range(N_CORES):
        arr = results[c]["out"]                     # [4, T1*32]
        out[1:, c * BS:(c + 1) * BS, :] = (
            arr.reshape(D, T1, BS).transpose(1, 2, 0))
    return out


_CACHE = {}


def kernel(s_grid, y0, W1, b1, W2, b2, W3, b3, wpoly):
    """Full-input, full-output entry point. Returns [T, 256, 4] float32."""
    import os
    os.environ.setdefault("NEURON_RT_RESET_CORES", "1")
    hs, in_maps = prep_inputs_dp(s_grid, y0, W1, b1, W2, b2, W3, b3, wpoly)
    key = tuple(np.asarray(hs, np.float64).round(12).tolist())
    if key not in _CACHE:
        _CACHE[key] = build_dp(hs)
    nc = _CACHE[key]
    from concourse import bass_utils
    res = None
    for attempt in range(3):
        try:
            res = bass_utils.run_bass_kernel_spmd(
                nc, in_maps, core_ids=list(range(N_CORES)))
            break
        except Exception:
            if attempt == 2:
                raise
    results = {c: res.results[c] for c in range(N_CORES)}
    return assemble_dp(results, y0).astype(np.float32)
